# revision 49
# baseline (speedup 1.0000x reference)
"""BrainGCN on 8 Trainium2 NeuronCores (Bass/Tile, SPMD).

kernel(**inputs) takes the FULL unsharded inputs and returns the full (G,)
output.  Internally:

Sharding: N nodes in 8 contiguous shards (SH=N/8); each edge is assigned to
the core owning its dst node, grouped into 128-node dst windows; within each
window edges are split by src parity (even/odd) and padded to 128-edge tiles;
per-(window,parity) tile counts are equalized across cores so one program
runs SPMD on all 8 cores.  Self-loops are excluded from the edge stream;
their contribution is applied per window as a diag(dinv) matmul against
locally retained rows.

Math: norm_e = dinv[src]*w_e*dinv[dst].
  L1: out1[d] = dinv[d] * (sum_e w'_e * x[src_e]) @ (W1*bns1) + c1; relu,
      with w'_e = w_e*dinv[src_e].  Aggregation runs in IN_DIM space from
      host-pre-gathered x[src_e] (pure input data movement).  dinv[src]
      comes from host-pre-gathered per-edge weight-slot rows (wde) that the
      device reduces + rsqrts (all FLOPs stay on device).
  L2: t2 = dinv * (h1 @ (W2*bns2)) per own shard (dinv[src] folded into the
      table), stored packed bf16 [SH,64]; one AllGather -> table [N,64] bf16
      (6.4MB, half the baseline's bytes).  Per-edge rows are fetched with
      bulk dma_gather (SWDGE, single_packet=False; >=1152 idxs/call faults
      with single_packet=True): edges are split by src%4 so int16 indices
      with 512B stride / 256B elem reach all 50k rows (the elem's second
      128B is a neighbor row, ignored by the 64-col rhs slice).
      out2[d] = dinv[d] * sum_e w_e * t2[src_e] + c2; relu.
Scatter-add = one-hot matmul into PSUM windows; one-hots are bf16, built on
DVE as (iota == dloc) * w with ~20%% offloaded to ACT as
relu(w - w*abs(iota - dloc)).  Pooling: one-hot(batch) matmul accumulated
over all windows plus a ones column for counts, AllReduce, tiny MLP head
replicated on every core; core 0's output is returned.
"""
import math
from contextlib import ExitStack

import numpy as np
import ml_dtypes

import concourse.bass as bass
import concourse.bacc as bacc
import concourse.tile as tile
import concourse.mybir as mybir
from concourse import library_config
from concourse.bass_utils import run_bass_kernel_spmd

F32 = mybir.dt.float32
BF16 = mybir.dt.bfloat16
I16 = mybir.dt.int16
AL = mybir.AluOpType
ACTF = mybir.ActivationFunctionType

N_CORES = 8
WIN = 128
EPS = 1e-5
GRP = 3          # windows per gather group
BF = ml_dtypes.bfloat16


def _prep_inputs(inputs: dict):
    x = np.asarray(inputs["x"], np.float32)
    ei = np.asarray(inputs["edge_index"])
    ew = np.asarray(inputs["edge_weight"], np.float32)
    batch = np.asarray(inputs["batch"]).astype(np.int64)
    N, IN_DIM = x.shape
    HID = np.asarray(inputs["W1"]).shape[1]
    assert N % N_CORES == 0 and N % 2 == 0
    SH = N // N_CORES
    NWIN = math.ceil(SH / WIN)

    # degree includes self-loop weight 1; the edge stream excludes self-loops.
    dstA = np.concatenate([np.asarray(ei[1]), np.arange(N)]).astype(np.int64)
    wA = np.concatenate([ew, np.ones(N, np.float32)]).astype(np.float32)
    orderA = np.argsort(dstA, kind="stable")
    dsA, wsA = dstA[orderA], wA[orderA]
    countsA = np.bincount(dstA, minlength=N)
    DSLOT = int(countsA.max())
    rowptrA = np.zeros(N + 1, np.int64)
    np.cumsum(countsA, out=rowptrA[1:])
    wdeg_full = np.zeros((N, DSLOT), np.float32)
    slotA = np.arange(len(dsA)) - rowptrA[dsA]
    wdeg_full[dsA, slotA] = wsA

    src = np.asarray(ei[0]).astype(np.int64)
    dst = np.asarray(ei[1]).astype(np.int64)
    w = ew.astype(np.float32)
    NPAR = 4   # parity classes (node id mod 4): int16 idx + 512B stride
    order = np.argsort(dst, kind="stable")
    ds, ss, ws = dst[order], src[order], w[order]
    ps = (ss % NPAR).astype(np.int64)
    counts = np.bincount(dst, minlength=N)
    rowptr = np.zeros(N + 1, np.int64)
    np.cumsum(counts, out=rowptr[1:])

    # per (core, window, parity) edge counts
    cntR = np.zeros((NPAR, N_CORES, NWIN), np.int64)
    seg = {}
    for c in range(N_CORES):
        for j in range(NWIN):
            lo = c * SH + j * WIN
            hi = min(c * SH + (j + 1) * WIN, (c + 1) * SH)
            a, b = int(rowptr[lo]), int(rowptr[hi])
            for r in range(NPAR):
                cntR[r, c, j] = int(np.count_nonzero(ps[a:b] == r))
            seg[(c, j)] = (lo, a, b)
    tilesR = [(-(-cntR[r].max(axis=0) // 128)).astype(np.int64)
              for r in range(NPAR)]
    TR = [int(t.sum()) for t in tilesR]
    TT = sum(TR)
    baseR = np.concatenate([[0], np.cumsum(TR)]).astype(int)
    colR = []
    for r in range(NPAR):
        cr = np.zeros(NWIN + 1, np.int64)
        np.cumsum(tilesR[r], out=cr[1:])
        colR.append(cr)

    # gather groups of GRP windows: per group, per parity tile ranges
    groups = []
    for g0 in range(0, NWIN, GRP):
        g1 = min(g0 + GRP, NWIN)
        groups.append((g0, g1,
                       [(int(colR[r][g0]), int(colR[r][g1]))
                        for r in range(NPAR)]))
    TGmax = [max(b - a for (_, _, rr) in groups for q, (a, b) in enumerate(rr)
                 if q == r) for r in range(NPAR)]

    # dense L1 layout (no parity constraint -> fewer pad tiles)
    cnt1 = np.zeros((N_CORES, NWIN), np.int64)
    for c in range(N_CORES):
        for j in range(NWIN):
            lo, a, b = seg[(c, j)]
            cnt1[c, j] = b - a
    tiles1 = (-(-cnt1.max(axis=0) // 128)).astype(np.int64)
    TT1 = int(tiles1.sum())
    col1 = np.zeros(NWIN + 1, np.int64)
    np.cumsum(tiles1, out=col1[1:])

    edloc = np.full((N_CORES, 128, TT), 999.0, np.float32)
    ewt = np.zeros((N_CORES, 128, TT), np.float32)
    idx16 = np.zeros((N_CORES, 16, TT * 8), np.int16)
    edloc1 = np.full((N_CORES, 128, TT1), 999.0, np.float32)
    ewt1 = np.zeros((N_CORES, 128, TT1), np.float32)
    xg = np.zeros((N_CORES, 128, TT1 * IN_DIM), BF)
    wde = np.zeros((N_CORES, 128, TT1 * DSLOT), BF)

    def pad_edges(ntiles, s_seg, d_seg, w_seg, lo):
        n = len(s_seg)
        cap = ntiles * 128
        s_pad = np.zeros(cap, np.int64)
        d_pad = np.full(cap, 999.0, np.float32)
        w_pad = np.zeros(cap, np.float32)
        s_pad[:n] = s_seg
        d_pad[:n] = (d_seg - lo).astype(np.float32)
        w_pad[:n] = w_seg
        return n, s_pad, d_pad, w_pad

    def fill_block2(c, tcol0, ntiles, s_seg, d_seg, w_seg, lo):
        n, s_pad, d_pad, w_pad = pad_edges(ntiles, s_seg, d_seg, w_seg, lo)
        edloc[c, :, tcol0:tcol0 + ntiles] = d_pad.reshape(ntiles, 128).T
        ewt[c, :, tcol0:tcol0 + ntiles] = w_pad.reshape(ntiles, 128).T
        # int16 gather indices: byte offset = idx*512 + parity*128
        iv = (s_pad // NPAR).astype(np.int16)
        iv[n:] = 0
        # edge (tile t local, partition p) at idx16[p%16, (tcol0+t)*8 + p//16]
        iv2 = iv.reshape(ntiles, 8, 16)          # [t, p//16, p%16]
        idx16[c, :, tcol0 * 8:(tcol0 + ntiles) * 8] = (
            iv2.transpose(2, 0, 1).reshape(16, ntiles * 8))

    def fill_block1(c, tcol0, ntiles, s_seg, d_seg, w_seg, lo):
        n, s_pad, d_pad, w_pad = pad_edges(ntiles, s_seg, d_seg, w_seg, lo)
        edloc1[c, :, tcol0:tcol0 + ntiles] = d_pad.reshape(ntiles, 128).T
        ewt1[c, :, tcol0:tcol0 + ntiles] = w_pad.reshape(ntiles, 128).T
        xs = x[s_pad]
        xs[n:] = 0.0
        xg[c, :, tcol0 * IN_DIM:(tcol0 + ntiles) * IN_DIM] = (
            xs.reshape(ntiles, 128, IN_DIM).transpose(1, 0, 2)
            .reshape(128, ntiles * IN_DIM).astype(BF))
        wd = wdeg_full[s_pad].copy()
        wd[n:] = 0.0
        wd[n:, 0] = 1.0
        wde[c, :, tcol0 * DSLOT:(tcol0 + ntiles) * DSLOT] = (
            wd.reshape(ntiles, 128, DSLOT).transpose(1, 0, 2)
            .reshape(128, ntiles * DSLOT).astype(BF))

    for c in range(N_CORES):
        for j in range(NWIN):
            lo, a, b = seg[(c, j)]
            for r in range(NPAR):
                m = ps[a:b] == r
                fill_block2(c, baseR[r] + int(colR[r][j]), int(tilesR[r][j]),
                            ss[a:b][m], ds[a:b][m], ws[a:b][m], lo)
            fill_block1(c, int(col1[j]), int(tiles1[j]),
                        ss[a:b], ds[a:b], ws[a:b], lo)

    PADN = NWIN * WIN

    def win_major(a2d, dt=np.float32):
        S = a2d.shape[1]
        assert a2d.shape[0] == PADN
        return np.ascontiguousarray(
            a2d.reshape(NWIN, WIN, S).transpose(1, 0, 2)
            .reshape(WIN, NWIN * S)).astype(dt)

    iota = np.tile(np.arange(128, dtype=np.float32), (128, 1))
    ident = np.eye(128, dtype=np.float32)

    in_maps = []
    for c in range(N_CORES):
        wc = np.zeros((PADN, DSLOT), np.float32)
        wc[:SH] = wdeg_full[c * SH:(c + 1) * SH]
        wc[SH:, 0] = 1.0  # pad nodes: deg=1 keeps rsqrt finite
        bv = np.full((PADN, 1), 999.0, np.float32)
        bv[:SH, 0] = batch[c * SH:(c + 1) * SH].astype(np.float32)
        xo = np.zeros((PADN, IN_DIM), np.float32)
        xo[:SH] = x[c * SH:(c + 1) * SH]
        in_maps.append({
            "edloc": edloc[c], "ewt": ewt[c],
            "edloc1": edloc1[c], "ewt1": ewt1[c],
            "xg": xg[c], "wde": wde[c],
            "idx16": np.tile(idx16[c], (8, 1)),
            "xnm": win_major(xo, BF),
            "wdeg": win_major(wc, BF), "batchv": win_major(bv),
            "iota": iota.astype(BF), "ident": ident,
            "identbf": ident.astype(BF),
            "W1": np.asarray(inputs["W1"], np.float32),
            "W2": np.asarray(inputs["W2"], np.float32),
            "g1": np.asarray(inputs["bn1_gamma"], np.float32).reshape(1, HID),
            "be1": np.asarray(inputs["bn1_beta"], np.float32).reshape(1, HID),
            "m1": np.asarray(inputs["bn1_mean"], np.float32).reshape(1, HID),
            "v1": np.asarray(inputs["bn1_var"], np.float32).reshape(1, HID),
            "b1": np.asarray(inputs["b1"], np.float32).reshape(1, HID),
            "g2": np.asarray(inputs["bn2_gamma"], np.float32).reshape(1, HID),
            "be2": np.asarray(inputs["bn2_beta"], np.float32).reshape(1, HID),
            "m2": np.asarray(inputs["bn2_mean"], np.float32).reshape(1, HID),
            "v2": np.asarray(inputs["bn2_var"], np.float32).reshape(1, HID),
            "b2": np.asarray(inputs["b2"], np.float32).reshape(1, HID),
            "lin1W": np.asarray(inputs["lin1_W"], np.float32),
            "lin1b": np.asarray(inputs["lin1_b"], np.float32).reshape(-1, 1),
            "lin2W": np.asarray(inputs["lin2_W"], np.float32),
            "lin2b": np.asarray(inputs["lin2_b"], np.float32).reshape(1, 1),
        })

    meta = dict(N=N, G=128, IN_DIM=IN_DIM, HID=HID, SH=SH, NWIN=NWIN,
                DSLOT=DSLOT, TT=TT, TT1=TT1, NPAR=NPAR,
                tiles1=[int(t) for t in tiles1],
                tilesR=[[int(t) for t in tr] for tr in tilesR],
                baseR=[int(b) for b in baseR],
                groups=groups, TGmax=TGmax)
    return in_maps, meta


def _build_nc(meta, reps=1, no_collectives=False, no_gather=False):
    N, IN_DIM, HID = meta["N"], meta["IN_DIM"], meta["HID"]
    SH, NWIN, DSLOT, TT = meta["SH"], meta["NWIN"], meta["DSLOT"], meta["TT"]
    NPAR, TT1 = meta["NPAR"], meta["TT1"]
    tiles1 = meta["tiles1"]
    col1 = np.concatenate([[0], np.cumsum(tiles1)]).astype(int)
    tilesR, baseR = meta["tilesR"], meta["baseR"]
    groups, TGmax = meta["groups"], meta["TGmax"]
    colR = [np.concatenate([[0], np.cumsum(tr)]).astype(int) for tr in tilesR]
    H2 = HID // 2

    nc = bacc.Bacc("TRN2", target_bir_lowering=False, debug=False,
                   num_devices=N_CORES)
    d_edloc = nc.dram_tensor("edloc", [128, TT], F32, kind="ExternalInput")
    d_ewt = nc.dram_tensor("ewt", [128, TT], F32, kind="ExternalInput")
    d_edloc1 = nc.dram_tensor("edloc1", [128, TT1], F32, kind="ExternalInput")
    d_ewt1 = nc.dram_tensor("ewt1", [128, TT1], F32, kind="ExternalInput")
    d_xg = nc.dram_tensor("xg", [128, TT1 * IN_DIM], BF16, kind="ExternalInput")
    d_wde = nc.dram_tensor("wde", [128, TT1 * DSLOT], BF16, kind="ExternalInput")
    d_idx = nc.dram_tensor("idx16", [128, TT * 8], I16, kind="ExternalInput")
    d_xnm = nc.dram_tensor("xnm", [128, NWIN * IN_DIM], BF16, kind="ExternalInput")
    d_wdeg = nc.dram_tensor("wdeg", [128, NWIN * DSLOT], BF16, kind="ExternalInput")
    d_batch = nc.dram_tensor("batchv", [128, NWIN], F32, kind="ExternalInput")
    d_iota = nc.dram_tensor("iota", [128, 128], BF16, kind="ExternalInput")
    d_ident = nc.dram_tensor("ident", [128, 128], F32, kind="ExternalInput")
    d_identbf = nc.dram_tensor("identbf", [128, 128], BF16, kind="ExternalInput")
    d_W1 = nc.dram_tensor("W1", [IN_DIM, HID], F32, kind="ExternalInput")
    d_W2 = nc.dram_tensor("W2", [HID, HID], F32, kind="ExternalInput")
    bn_names = ["g1", "be1", "m1", "v1", "b1", "g2", "be2", "m2", "v2", "b2"]
    d_bn = {k: nc.dram_tensor(k, [1, HID], F32, kind="ExternalInput")
            for k in bn_names}
    d_lin1W = nc.dram_tensor("lin1W", [HID, H2], F32, kind="ExternalInput")
    d_lin1b = nc.dram_tensor("lin1b", [H2, 1], F32, kind="ExternalInput")
    d_lin2W = nc.dram_tensor("lin2W", [H2, 1], F32, kind="ExternalInput")
    d_lin2b = nc.dram_tensor("lin2b", [1, 1], F32, kind="ExternalInput")
    d_out = nc.dram_tensor("out", [1, 128], F32, kind="ExternalOutput")

    rg = [list(range(N_CORES))]

    with tile.TileContext(nc) as tc, ExitStack() as ctx:
        constp = ctx.enter_context(tc.tile_pool(name="const", bufs=1))
        metap = ctx.enter_context(tc.tile_pool(name="meta", bufs=1))
        wdep = ctx.enter_context(tc.tile_pool(name="wdep", bufs=2))
        msgsp = ctx.enter_context(tc.tile_pool(name="msgs", bufs=2))
        ohp = ctx.enter_context(tc.tile_pool(name="oh", bufs=12))
        ohgp = ctx.enter_context(tc.tile_pool(name="ohg", bufs=4))
        epp = ctx.enter_context(tc.tile_pool(name="ep", bufs=4))
        vecp = ctx.enter_context(tc.tile_pool(name="vec", bufs=1))
        psA = ctx.enter_context(tc.tile_pool(name="psA", bufs=2, space="PSUM"))
        ps5 = ctx.enter_context(tc.tile_pool(name="ps5", bufs=2, space="PSUM"))
        psB = ctx.enter_context(tc.tile_pool(name="psB", bufs=2, space="PSUM"))
        psPool = ctx.enter_context(tc.tile_pool(name="psP", bufs=1, space="PSUM"))
        dram = ctx.enter_context(tc.tile_pool(name="dram", bufs=1, space="DRAM"))

        nc.gpsimd.load_library(library_config.mlp)
        iota = constp.tile([128, 128], BF16)
        nc.sync.dma_start(iota[:], d_iota.ap())
        ident = constp.tile([128, 128], F32)
        nc.sync.dma_start(ident[:], d_ident.ap())
        identbf = constp.tile([128, 128], BF16)
        nc.sync.dma_start(identbf[:], d_identbf.ap())
        ones1 = constp.tile([1, 128], F32)
        nc.vector.memset(ones1[:], 1.0)

        sb_edloc = metap.tile([128, TT], F32)
        sb_ewt = metap.tile([128, TT], F32)
        sb_edloc1 = metap.tile([128, TT1], F32)
        sb_ewt1 = metap.tile([128, TT1], F32)
        nc.sync.dma_start(sb_edloc1[:], d_edloc1.ap())
        nc.sync.dma_start(sb_ewt1[:], d_ewt1.ap())
        sb_batch = metap.tile([128, NWIN], F32)
        sb_xg = metap.tile([128, TT1 * IN_DIM], BF16)
        sb_xnm = metap.tile([128, NWIN * IN_DIM], BF16)
        sb_idx = metap.tile([128, TT * 8], I16)
        nc.sync.dma_start(sb_xnm[:], d_xnm.ap())
        t2keep = metap.tile([128, NWIN * HID], BF16)
        nc.sync.dma_start(sb_edloc[:], d_edloc.ap())
        nc.sync.dma_start(sb_ewt[:], d_ewt.ap())
        nc.sync.dma_start(sb_batch[:], d_batch.ap())
        nc.sync.dma_start(sb_xg[:], d_xg.ap())
        nc.sync.dma_start(sb_idx[:], d_idx.ap())
        sb_W1 = constp.tile([IN_DIM, HID], F32)
        sb_W2 = constp.tile([HID, HID], F32)
        nc.sync.dma_start(sb_W1[:], d_W1.ap())
        nc.sync.dma_start(sb_W2[:], d_W2.ap())
        sb_bn = {}
        for k in bn_names:
            sb_bn[k] = vecp.tile([1, HID], F32, tag=k, name="sb_" + k)
            nc.sync.dma_start(sb_bn[k][:], d_bn[k].ap())
        sb_lin1W = constp.tile([HID, H2], F32)
        sb_lin1b = constp.tile([H2, 1], F32)
        sb_lin2W = constp.tile([H2, 1], F32)
        sb_lin2b = constp.tile([1, 1], F32)
        nc.sync.dma_start(sb_lin1W[:], d_lin1W.ap())
        nc.sync.dma_start(sb_lin1b[:], d_lin1b.ap())
        nc.sync.dma_start(sb_lin2W[:], d_lin2W.ap())
        nc.sync.dma_start(sb_lin2b[:], d_lin2b.ap())

        # BN folds: bns = gamma*rsqrt(var+eps); c = bns*(b - mean) + beta
        def bn_fold(g, be, m, v, b):
            bns = vecp.tile([1, HID], F32, tag="bns" + g, name="bns" + g)
            nc.vector.tensor_scalar(out=bns[:], in0=sb_bn[v][:], scalar1=EPS,
                                    scalar2=None, op0=AL.add)
            nc.scalar.activation(bns[:], bns[:], ACTF.Sqrt)
            nc.vector.reciprocal(bns[:], bns[:])
            nc.vector.tensor_tensor(out=bns[:], in0=bns[:], in1=sb_bn[g][:],
                                    op=AL.mult)
            cc = vecp.tile([1, HID], F32, tag="c" + g, name="c" + g)
            nc.vector.tensor_tensor(out=cc[:], in0=sb_bn[b][:], in1=sb_bn[m][:],
                                    op=AL.subtract)
            nc.vector.tensor_tensor(out=cc[:], in0=cc[:], in1=bns[:], op=AL.mult)
            nc.vector.tensor_tensor(out=cc[:], in0=cc[:], in1=sb_bn[be][:],
                                    op=AL.add)
            return bns, cc

        bns1, c1v = bn_fold("g1", "be1", "m1", "v1", "b1")
        bns2, c2v = bn_fold("g2", "be2", "m2", "v2", "b2")

        def bcast128(vec, tag):
            ps = psB.tile([128, HID], F32, tag="B", name="bc" + tag)
            nc.tensor.matmul(out=ps[:], lhsT=ones1[:], rhs=vec[:],
                             start=True, stop=True)
            sb = constp.tile([128, HID], F32, tag=tag, name="sb" + tag)
            nc.vector.tensor_copy(sb[:], ps[:])
            return sb

        c1_b = bcast128(c1v, "c1b")
        c2_b = bcast128(c2v, "c2b")

        def wfold(sb_W, bns, parts, tag):
            one_r = constp.tile([1, parts], F32, tag="oner" + tag,
                                name="oner" + tag)
            nc.vector.memset(one_r[:], 1.0)
            ps = psB.tile([parts, HID], F32, tag="B", name="wf" + tag)
            nc.tensor.matmul(out=ps[:], lhsT=one_r[:], rhs=bns[:],
                             start=True, stop=True)
            wp = constp.tile([parts, HID], F32, tag="wp" + tag, name="wp" + tag)
            nc.vector.tensor_tensor(out=wp[:], in0=sb_W[:], in1=ps[:], op=AL.mult)
            return wp

        W1p = wfold(sb_W1, bns1, IN_DIM, "1")
        W2p = wfold(sb_W2, bns2, HID, "2")

        t2_sh = dram.tile([SH, HID], BF16)
        t2_full = dram.tile([N + NPAR, HID], BF16)
        t2flat = t2_full[:].rearrange("n h -> (n h)")
        NROW4 = N // NPAR
        t2vr = [t2flat[r * HID: r * HID + NROW4 * NPAR * HID]
                .rearrange("(m k) -> m k", k=NPAR * HID)
                for r in range(NPAR)]
        zrow = constp.tile([NPAR, HID], BF16)
        nc.vector.memset(zrow[:], 0.0)

        dinv = constp.tile([128, NWIN], F32)
        wprime = constp.tile([128, TT1], F32)
        negwp = constp.tile([128, TT1], F32)
        negdloc1 = constp.tile([128, TT1], F32)
        negdloc = constp.tile([128, TT], F32)
        negewt = constp.tile([128, TT], F32)
        sb_wdeg = metap.tile([128, NWIN * DSLOT], BF16)

        for rep in range(reps):
            nc.sync.dma_start(t2_full[N:N + NPAR, :], zrow[:])

            # own-shard degree -> dinv_d [128, NWIN]
            nc.sync.dma_start(sb_wdeg[:], d_wdeg.ap())
            nc.vector.tensor_reduce(
                out=dinv[:].rearrange("p (j s) -> p j s", s=1),
                in_=sb_wdeg[:].rearrange("p (j s) -> p j s", s=DSLOT),
                op=AL.add, axis=mybir.AxisListType.X)
            nc.scalar.activation(dinv[:], dinv[:], ACTF.Sqrt)
            nc.vector.reciprocal(dinv[:], dinv[:])

            # per-edge w' = ewt * rsqrt(deg[src]) via chunked wde reduction
            CH = 64
            for c0 in range(0, TT1, CH):
                cw = min(CH, TT1 - c0)
                wchunk = wdep.tile([128, CH * DSLOT], BF16, tag="wde",
                                   name="wchunk")
                nc.sync.dma_start(wchunk[:, :cw * DSLOT],
                                  d_wde.ap()[:, c0 * DSLOT:(c0 + cw) * DSLOT])
                nc.vector.tensor_reduce(
                    out=wprime[:, c0:c0 + cw].rearrange("p (j s) -> p j s", s=1),
                    in_=wchunk[:, :cw * DSLOT].rearrange("p (j s) -> p j s",
                                                         s=DSLOT),
                    op=AL.add, axis=mybir.AxisListType.X)
                nc.scalar.activation(wprime[:, c0:c0 + cw], wprime[:, c0:c0 + cw],
                                     ACTF.Sqrt)
                nc.vector.reciprocal(wprime[:, c0:c0 + cw], wprime[:, c0:c0 + cw])
                nc.vector.tensor_tensor(out=wprime[:, c0:c0 + cw],
                                        in0=wprime[:, c0:c0 + cw],
                                        in1=sb_ewt1[:, c0:c0 + cw], op=AL.mult)

            nc.vector.tensor_scalar(out=negdloc[:], in0=sb_edloc[:],
                                    scalar1=-1.0, scalar2=None, op0=AL.mult)
            nc.vector.tensor_scalar(out=negdloc1[:], in0=sb_edloc1[:],
                                    scalar1=-1.0, scalar2=None, op0=AL.mult)
            nc.vector.tensor_scalar(out=negewt[:], in0=sb_ewt[:],
                                    scalar1=-1.0, scalar2=None, op0=AL.mult)
            nc.vector.tensor_scalar(out=negwp[:], in0=wprime[:],
                                    scalar1=-1.0, scalar2=None, op0=AL.mult)

            def build_diag(j):
                dg = ohp.tile([128, 128], BF16, tag="oh", name="dg")
                nc.vector.tensor_scalar(
                    out=dg[:], in0=ident[:], scalar1=dinv[:, j:j + 1],
                    scalar2=None, op0=AL.mult)
                return dg

            def oh_into(dst, col, dloctile, negdloctile, wtile, negwtile,
                        on_act):
                if on_act:
                    tt = ohp.tile([128, 128], BF16, tag="att", name="att")
                    nc.scalar.activation(tt[:], iota[:], ACTF.Abs,
                                         bias=negdloctile[:, col:col + 1])
                    nc.scalar.activation(dst, tt[:], ACTF.Relu,
                                         bias=wtile[:, col:col + 1],
                                         scale=negwtile[:, col:col + 1])
                else:
                    nc.vector.tensor_scalar(
                        out=dst, in0=iota[:],
                        scalar1=dloctile[:, col:col + 1],
                        scalar2=wtile[:, col:col + 1],
                        op0=AL.is_equal, op1=AL.mult)

            def build_onehot(col, on_act=False):
                oh = ohp.tile([128, 128], BF16, tag="oh", name="oh")
                oh_into(oh[:], col, sb_edloc1, negdloc1, wprime, negwp, on_act)
                return oh

            # L1
            for j in range(NWIN):
                wlen = min(WIN, SH - j * WIN)
                acc5 = ps5.tile([IN_DIM, 128], F32, tag="acc5", name="acc5")
                cols = [int(col1[j]) + t for t in range(tiles1[j])]
                for k, col in enumerate(cols):
                    oh = build_onehot(col, on_act=(k % 7 == 6))
                    nc.tensor.matmul(
                        out=acc5[:],
                        lhsT=sb_xg[:, col * IN_DIM:(col + 1) * IN_DIM],
                        rhs=oh[:], start=(k == 0), stop=False)
                dg1 = build_diag(j)
                nc.tensor.matmul(
                    out=acc5[:],
                    lhsT=sb_xnm[:, j * IN_DIM:(j + 1) * IN_DIM],
                    rhs=dg1[:], start=False, stop=True)
                agg5 = epp.tile([IN_DIM, 128], F32, tag="agg5", name="agg5")
                nc.vector.tensor_copy(agg5[:], acc5[:])
                ps1 = psB.tile([128, HID], F32, tag="B", name="ps1")
                nc.tensor.matmul(out=ps1[:], lhsT=agg5[:], rhs=W1p[:],
                                 start=True, stop=True)
                h1 = epp.tile([128, HID], F32, tag="h1", name="h1")
                nc.vector.scalar_tensor_tensor(
                    out=h1[:], in0=ps1[:], scalar=dinv[:, j:j + 1],
                    in1=c1_b[:], op0=AL.mult, op1=AL.add)
                nc.scalar.activation(h1[:], h1[:], ACTF.Relu)
                pT = psB.tile([HID, 128], F32, tag="B", name="pT")
                nc.tensor.transpose(out=pT[:], in_=h1[:], identity=ident[:])
                h1T = epp.tile([HID, 128], F32, tag="h1T", name="h1T")
                nc.vector.tensor_copy(h1T[:], pT[:])
                ps2 = psB.tile([128, HID], F32, tag="B", name="ps2")
                nc.tensor.matmul(out=ps2[:], lhsT=h1T[:], rhs=W2p[:],
                                 start=True, stop=True)
                # fold dinv into the table row; write bf16 into padded layout
                nc.vector.tensor_scalar(
                    out=t2keep[:, j * HID:(j + 1) * HID], in0=ps2[:],
                    scalar1=dinv[:, j:j + 1], scalar2=None, op0=AL.mult)
                nc.sync.dma_start(t2_sh[j * WIN:j * WIN + wlen, :],
                                  t2keep[:wlen, j * HID:(j + 1) * HID])



            if not no_collectives:
                nc.gpsimd.collective_compute(
                    "AllGather", AL.bypass, replica_groups=rg,
                    ins=[t2_sh[:]], outs=[t2_full[0:N, :]])

            # L2: bulk parity gathers + one-hot scatter + pooling
            pool_ps = psPool.tile([128, HID + 1], F32)
            for (g0, g1, rr) in groups:
                nR = [b - a for (a, b) in rr]
                gbase = np.concatenate([[0], np.cumsum(nR)]).astype(int)
                GT = int(gbase[-1])
                msgsG = msgsp.tile([128, sum(TGmax) * 128], BF16, tag="mG",
                                   name="msgsG")
                for r in range(NPAR):
                    a, b = rr[r]
                    if a == b:
                        continue
                    mslice = msgsG[:, int(gbase[r]) * 128:int(gbase[r + 1]) * 128]
                    if no_gather:
                        nc.vector.memset(mslice, 0.0)
                    else:
                        nc.gpsimd.dma_gather(
                            out_ap=mslice.rearrange("p (t h) -> p t h", h=128),
                            in_ap=t2vr[r][:, 0:128],
                            idxs_ap=sb_idx[:, (baseR[r] + a) * 8:
                                           (baseR[r] + b) * 8],
                            num_idxs=(b - a) * 128, num_idxs_reg=(b - a) * 128,
                            elem_size=128, elem_step=2 * 128,
                            single_packet=False)
                ohG = ohgp.tile([128, sum(TGmax) * 128], BF16,
                                tag="ohG", name="ohG")
                for j in range(g0, g1):
                    for r in range(NPAR):
                        a, _ = rr[r]
                        for t in range(tilesR[r][j]):
                            col = baseR[r] + int(colR[r][j]) + t
                            k = int(gbase[r]) + int(colR[r][j]) - a + t
                            oh_into(ohG[:, k * 128:(k + 1) * 128], col,
                                    sb_edloc, negdloc, sb_ewt, negewt,
                                    on_act=(k % 5 == 4))
                for j in range(g0, g1):
                    acc = psA.tile([128, HID], F32, tag="acc", name="acc")
                    first = True
                    for r in range(NPAR):
                        a, _ = rr[r]
                        for t in range(tilesR[r][j]):
                            k = int(gbase[r]) + int(colR[r][j]) - a + t
                            mc = k * 128
                            nc.tensor.matmul(out=acc[:],
                                             lhsT=ohG[:, mc:mc + 128],
                                             rhs=msgsG[:, mc:mc + HID],
                                             start=first, stop=False)
                            first = False
                    nc.tensor.matmul(out=acc[:], lhsT=identbf[:],
                                     rhs=t2keep[:, j * HID:(j + 1) * HID],
                                     start=False, stop=True)
                    h2e = epp.tile([128, HID + 1], BF16, tag="h2e", name="h2e")
                    nc.vector.scalar_tensor_tensor(
                        out=h2e[:, :HID], in0=acc[:], scalar=dinv[:, j:j + 1],
                        in1=c2_b[:], op0=AL.mult, op1=AL.add)
                    nc.scalar.activation(h2e[:, :HID], h2e[:, :HID], ACTF.Relu)
                    nc.vector.memset(h2e[:, HID:], 1.0)
                    ohb = ohp.tile([128, 128], BF16, tag="ohb", name="ohb")
                    nc.vector.tensor_scalar(out=ohb[:], in0=iota[:],
                                            scalar1=sb_batch[:, j:j + 1],
                                            scalar2=None, op0=AL.is_equal)
                    nc.tensor.matmul(out=pool_ps[:], lhsT=ohb[:], rhs=h2e[:],
                                     start=(j == 0), stop=(j == NWIN - 1),
                                     skip_group_check=True)

            pool_sb = epp.tile([128, HID + 1], F32, tag="poolsb", name="pool_sb")
            nc.vector.tensor_copy(pool_sb[:], pool_ps[:])
            ar_in = dram.tile([128, HID + 1], F32)
            ar_out = dram.tile([128, HID + 1], F32)
            nc.sync.dma_start(ar_in[:], pool_sb[:])
            if no_collectives:
                nc.sync.dma_start(ar_out[:], ar_in[:])
            else:
                nc.gpsimd.collective_compute(
                    "AllReduce", AL.add, replica_groups=rg,
                    ins=[ar_in.opt()], outs=[ar_out.opt()])
            sums = epp.tile([128, HID + 1], F32, tag="sums", name="sums")
            nc.sync.dma_start(sums[:], ar_out[:])

            cntc = epp.tile([128, 1], F32, tag="cnt", name="cntc")
            nc.vector.tensor_scalar(out=cntc[:], in0=sums[:, HID:HID + 1],
                                    scalar1=1.0, scalar2=None, op0=AL.max)
            rc = epp.tile([128, 1], F32, tag="rc", name="rc")
            nc.vector.reciprocal(rc[:], cntc[:])
            pooled = epp.tile([128, HID], F32, tag="pooled", name="pooled")
            nc.vector.tensor_scalar(out=pooled[:], in0=sums[:, :HID],
                                    scalar1=rc[:, :1], scalar2=None, op0=AL.mult)
            pT2 = psB.tile([HID, 128], F32, tag="B", name="pT2")
            nc.tensor.transpose(out=pT2[:], in_=pooled[:], identity=ident[:])
            pooledT = epp.tile([HID, 128], F32, tag="pooledT", name="pooledT")
            nc.vector.tensor_copy(pooledT[:], pT2[:])
            zps = psB.tile([H2, 128], F32, tag="B", name="zps")
            nc.tensor.matmul(out=zps[:], lhsT=sb_lin1W[:], rhs=pooledT[:],
                             start=True, stop=True)
            zT = epp.tile([H2, 128], F32, tag="zT", name="zT")
            nc.scalar.activation(zT[:], zps[:], ACTF.Relu, bias=sb_lin1b[:, :1])
            ops = psB.tile([1, 128], F32, tag="B", name="ops")
            nc.tensor.matmul(out=ops[:], lhsT=sb_lin2W[:], rhs=zT[:],
                             start=True, stop=True)
            outsb = epp.tile([1, 128], F32, tag="outsb", name="outsb")
            nc.vector.tensor_scalar(out=outsb[:], in0=ops[:],
                                    scalar1=sb_lin2b[:, :1], scalar2=None,
                                    op0=AL.add)
            nc.sync.dma_start(d_out.ap(), outsb[:])

    nc.compile()
    return nc


_CACHE = {}


def kernel(**inputs) -> np.ndarray:
    in_maps, meta = _prep_inputs(inputs)
    key = (meta["N"], meta["TT"], meta["TT1"], meta["DSLOT"],
           tuple(tuple(tr) for tr in meta["tilesR"]), tuple(meta["tiles1"]))
    if key not in _CACHE:
        _CACHE[key] = _build_nc(meta)
    nc = _CACHE[key]
    res = run_bass_kernel_spmd(nc, in_maps, core_ids=list(range(N_CORES)))
    out = np.asarray(res.results[0]["out"], np.float32).reshape(-1)
    return out[:meta["G"]].copy()


# revision 52
# speedup vs baseline: 1.0105x; 1.0105x over previous
"""BrainGCN on 8 Trainium2 NeuronCores (Bass/Tile, SPMD).

kernel(**inputs) takes the FULL unsharded inputs and returns the full (G,)
output.  Internally:

Sharding: N nodes in 8 contiguous shards (SH=N/8); each edge is assigned to
the core owning its dst node, grouped into 128-node dst windows; within each
window edges are split by src parity (even/odd) and padded to 128-edge tiles;
per-(window,parity) tile counts are equalized across cores so one program
runs SPMD on all 8 cores.  Self-loops are excluded from the edge stream;
their contribution is applied per window as a diag(dinv) matmul against
locally retained rows.

Math: norm_e = dinv[src]*w_e*dinv[dst].
  L1: out1[d] = dinv[d] * (sum_e w'_e * x[src_e]) @ (W1*bns1) + c1; relu,
      with w'_e = w_e*dinv[src_e].  Aggregation runs in IN_DIM space from
      host-pre-gathered x[src_e] (pure input data movement).  dinv[src]
      comes from host-pre-gathered per-edge weight-slot rows (wde) that the
      device reduces + rsqrts (all FLOPs stay on device).
  L2: t2 = dinv * (h1 @ (W2*bns2)) per own shard (dinv[src] folded into the
      table), stored packed bf16 [SH,64]; one AllGather -> table [N,64] bf16
      (6.4MB, half the baseline's bytes).  Per-edge rows are fetched with
      bulk dma_gather (SWDGE, single_packet=False; >=1152 idxs/call faults
      with single_packet=True): edges are split by src%4 so int16 indices
      with 512B stride / 256B elem reach all 50k rows (the elem's second
      128B is a neighbor row, ignored by the 64-col rhs slice).
      out2[d] = dinv[d] * sum_e w_e * t2[src_e] + c2; relu.
Scatter-add = one-hot matmul into PSUM windows; one-hots are bf16, built on
DVE as (iota == dloc) * w with ~20%% offloaded to ACT as
relu(w - w*abs(iota - dloc)).  Pooling: one-hot(batch) matmul accumulated
over all windows plus a ones column for counts, AllReduce, tiny MLP head
replicated on every core; core 0's output is returned.
"""
import math
from contextlib import ExitStack

import numpy as np
import ml_dtypes

import concourse.bass as bass
import concourse.bacc as bacc
import concourse.tile as tile
import concourse.mybir as mybir
from concourse import library_config
from concourse.bass_utils import run_bass_kernel_spmd

F32 = mybir.dt.float32
BF16 = mybir.dt.bfloat16
I16 = mybir.dt.int16
AL = mybir.AluOpType
ACTF = mybir.ActivationFunctionType

N_CORES = 8
WIN = 128
EPS = 1e-5
GRP = 3          # windows per gather group
BF = ml_dtypes.bfloat16


def _prep_inputs(inputs: dict):
    x = np.asarray(inputs["x"], np.float32)
    ei = np.asarray(inputs["edge_index"])
    ew = np.asarray(inputs["edge_weight"], np.float32)
    batch = np.asarray(inputs["batch"]).astype(np.int64)
    N, IN_DIM = x.shape
    HID = np.asarray(inputs["W1"]).shape[1]
    assert N % N_CORES == 0 and N % 2 == 0
    SH = N // N_CORES
    NWIN = math.ceil(SH / WIN)

    # degree includes self-loop weight 1; the edge stream excludes self-loops.
    dstA = np.concatenate([np.asarray(ei[1]), np.arange(N)]).astype(np.int64)
    wA = np.concatenate([ew, np.ones(N, np.float32)]).astype(np.float32)
    orderA = np.argsort(dstA, kind="stable")
    dsA, wsA = dstA[orderA], wA[orderA]
    countsA = np.bincount(dstA, minlength=N)
    DSLOT = int(countsA.max())
    rowptrA = np.zeros(N + 1, np.int64)
    np.cumsum(countsA, out=rowptrA[1:])
    wdeg_full = np.zeros((N, DSLOT), np.float32)
    slotA = np.arange(len(dsA)) - rowptrA[dsA]
    wdeg_full[dsA, slotA] = wsA

    src = np.asarray(ei[0]).astype(np.int64)
    dst = np.asarray(ei[1]).astype(np.int64)
    w = ew.astype(np.float32)
    NPAR = 4   # parity classes (node id mod 4): int16 idx + 512B stride
    order = np.argsort(dst, kind="stable")
    ds, ss, ws = dst[order], src[order], w[order]
    ps = (ss % NPAR).astype(np.int64)
    counts = np.bincount(dst, minlength=N)
    rowptr = np.zeros(N + 1, np.int64)
    np.cumsum(counts, out=rowptr[1:])

    # per (core, window, parity) edge counts
    cntR = np.zeros((NPAR, N_CORES, NWIN), np.int64)
    seg = {}
    for c in range(N_CORES):
        for j in range(NWIN):
            lo = c * SH + j * WIN
            hi = min(c * SH + (j + 1) * WIN, (c + 1) * SH)
            a, b = int(rowptr[lo]), int(rowptr[hi])
            for r in range(NPAR):
                cntR[r, c, j] = int(np.count_nonzero(ps[a:b] == r))
            seg[(c, j)] = (lo, a, b)
    tilesR = [(-(-cntR[r].max(axis=0) // 128)).astype(np.int64)
              for r in range(NPAR)]
    TR = [int(t.sum()) for t in tilesR]
    TT = sum(TR)
    baseR = np.concatenate([[0], np.cumsum(TR)]).astype(int)
    colR = []
    for r in range(NPAR):
        cr = np.zeros(NWIN + 1, np.int64)
        np.cumsum(tilesR[r], out=cr[1:])
        colR.append(cr)

    # gather groups of GRP windows: per group, per parity tile ranges
    groups = []
    for g0 in range(0, NWIN, GRP):
        g1 = min(g0 + GRP, NWIN)
        groups.append((g0, g1,
                       [(int(colR[r][g0]), int(colR[r][g1]))
                        for r in range(NPAR)]))
    TGmax = [max(b - a for (_, _, rr) in groups for q, (a, b) in enumerate(rr)
                 if q == r) for r in range(NPAR)]

    # dense L1 layout (no parity constraint -> fewer pad tiles)
    cnt1 = np.zeros((N_CORES, NWIN), np.int64)
    for c in range(N_CORES):
        for j in range(NWIN):
            lo, a, b = seg[(c, j)]
            cnt1[c, j] = b - a
    tiles1 = (-(-cnt1.max(axis=0) // 128)).astype(np.int64)
    TT1 = int(tiles1.sum())
    col1 = np.zeros(NWIN + 1, np.int64)
    np.cumsum(tiles1, out=col1[1:])

    edloc = np.full((N_CORES, 128, TT), 999.0, np.float32)
    ewt = np.zeros((N_CORES, 128, TT), np.float32)
    idx16 = np.zeros((N_CORES, 16, TT * 8), np.int16)
    edloc1 = np.full((N_CORES, 128, TT1), 999.0, np.float32)
    ewt1 = np.zeros((N_CORES, 128, TT1), np.float32)
    xg = np.zeros((N_CORES, 128, TT1 * IN_DIM), BF)
    wde = np.zeros((N_CORES, 128, TT1 * DSLOT), BF)

    def pad_edges(ntiles, s_seg, d_seg, w_seg, lo):
        n = len(s_seg)
        cap = ntiles * 128
        s_pad = np.zeros(cap, np.int64)
        d_pad = np.full(cap, 999.0, np.float32)
        w_pad = np.zeros(cap, np.float32)
        s_pad[:n] = s_seg
        d_pad[:n] = (d_seg - lo).astype(np.float32)
        w_pad[:n] = w_seg
        return n, s_pad, d_pad, w_pad

    def fill_block2(c, tcol0, ntiles, s_seg, d_seg, w_seg, lo):
        n, s_pad, d_pad, w_pad = pad_edges(ntiles, s_seg, d_seg, w_seg, lo)
        edloc[c, :, tcol0:tcol0 + ntiles] = d_pad.reshape(ntiles, 128).T
        ewt[c, :, tcol0:tcol0 + ntiles] = w_pad.reshape(ntiles, 128).T
        # int16 gather indices: byte offset = idx*512 + parity*128
        iv = (s_pad // NPAR).astype(np.int16)
        iv[n:] = 0
        # edge (tile t local, partition p) at idx16[p%16, (tcol0+t)*8 + p//16]
        iv2 = iv.reshape(ntiles, 8, 16)          # [t, p//16, p%16]
        idx16[c, :, tcol0 * 8:(tcol0 + ntiles) * 8] = (
            iv2.transpose(2, 0, 1).reshape(16, ntiles * 8))

    def fill_block1(c, tcol0, ntiles, s_seg, d_seg, w_seg, lo):
        n, s_pad, d_pad, w_pad = pad_edges(ntiles, s_seg, d_seg, w_seg, lo)
        edloc1[c, :, tcol0:tcol0 + ntiles] = d_pad.reshape(ntiles, 128).T
        ewt1[c, :, tcol0:tcol0 + ntiles] = w_pad.reshape(ntiles, 128).T
        xs = x[s_pad]
        xs[n:] = 0.0
        xg[c, :, tcol0 * IN_DIM:(tcol0 + ntiles) * IN_DIM] = (
            xs.reshape(ntiles, 128, IN_DIM).transpose(1, 0, 2)
            .reshape(128, ntiles * IN_DIM).astype(BF))
        wd = wdeg_full[s_pad].copy()
        wd[n:] = 0.0
        wd[n:, 0] = 1.0
        wde[c, :, tcol0 * DSLOT:(tcol0 + ntiles) * DSLOT] = (
            wd.reshape(ntiles, 128, DSLOT).transpose(1, 0, 2)
            .reshape(128, ntiles * DSLOT).astype(BF))

    for c in range(N_CORES):
        for j in range(NWIN):
            lo, a, b = seg[(c, j)]
            for r in range(NPAR):
                m = ps[a:b] == r
                fill_block2(c, baseR[r] + int(colR[r][j]), int(tilesR[r][j]),
                            ss[a:b][m], ds[a:b][m], ws[a:b][m], lo)
            fill_block1(c, int(col1[j]), int(tiles1[j]),
                        ss[a:b], ds[a:b], ws[a:b], lo)

    PADN = NWIN * WIN

    def win_major(a2d, dt=np.float32):
        S = a2d.shape[1]
        assert a2d.shape[0] == PADN
        return np.ascontiguousarray(
            a2d.reshape(NWIN, WIN, S).transpose(1, 0, 2)
            .reshape(WIN, NWIN * S)).astype(dt)

    iota = np.tile(np.arange(128, dtype=np.float32), (128, 1))
    ident = np.eye(128, dtype=np.float32)

    in_maps = []
    for c in range(N_CORES):
        wc = np.zeros((PADN, DSLOT), np.float32)
        wc[:SH] = wdeg_full[c * SH:(c + 1) * SH]
        wc[SH:, 0] = 1.0  # pad nodes: deg=1 keeps rsqrt finite
        bv = np.full((PADN, 1), 999.0, np.float32)
        bv[:SH, 0] = batch[c * SH:(c + 1) * SH].astype(np.float32)
        xo = np.zeros((PADN, IN_DIM), np.float32)
        xo[:SH] = x[c * SH:(c + 1) * SH]
        in_maps.append({
            "edloc": edloc[c], "ewt": ewt[c],
            "edloc1": edloc1[c], "ewt1": ewt1[c],
            "xg": xg[c], "wde": wde[c],
            "idx16": np.tile(idx16[c], (8, 1)),
            "xnm": win_major(xo, BF),
            "wdeg": win_major(wc, BF), "batchv": win_major(bv),
            "iota": iota.astype(BF), "ident": ident,
            "identbf": ident.astype(BF),
            "W1": np.asarray(inputs["W1"], np.float32),
            "W2": np.asarray(inputs["W2"], np.float32),
            "g1": np.asarray(inputs["bn1_gamma"], np.float32).reshape(1, HID),
            "be1": np.asarray(inputs["bn1_beta"], np.float32).reshape(1, HID),
            "m1": np.asarray(inputs["bn1_mean"], np.float32).reshape(1, HID),
            "v1": np.asarray(inputs["bn1_var"], np.float32).reshape(1, HID),
            "b1": np.asarray(inputs["b1"], np.float32).reshape(1, HID),
            "g2": np.asarray(inputs["bn2_gamma"], np.float32).reshape(1, HID),
            "be2": np.asarray(inputs["bn2_beta"], np.float32).reshape(1, HID),
            "m2": np.asarray(inputs["bn2_mean"], np.float32).reshape(1, HID),
            "v2": np.asarray(inputs["bn2_var"], np.float32).reshape(1, HID),
            "b2": np.asarray(inputs["b2"], np.float32).reshape(1, HID),
            "lin1W": np.asarray(inputs["lin1_W"], np.float32),
            "lin1b": np.asarray(inputs["lin1_b"], np.float32).reshape(-1, 1),
            "lin2W": np.asarray(inputs["lin2_W"], np.float32),
            "lin2b": np.asarray(inputs["lin2_b"], np.float32).reshape(1, 1),
        })

    meta = dict(N=N, G=128, IN_DIM=IN_DIM, HID=HID, SH=SH, NWIN=NWIN,
                DSLOT=DSLOT, TT=TT, TT1=TT1, NPAR=NPAR,
                tiles1=[int(t) for t in tiles1],
                tilesR=[[int(t) for t in tr] for tr in tilesR],
                baseR=[int(b) for b in baseR],
                groups=groups, TGmax=TGmax)
    return in_maps, meta


def _build_nc(meta, reps=1, no_collectives=False, no_gather=False):
    N, IN_DIM, HID = meta["N"], meta["IN_DIM"], meta["HID"]
    SH, NWIN, DSLOT, TT = meta["SH"], meta["NWIN"], meta["DSLOT"], meta["TT"]
    NPAR, TT1 = meta["NPAR"], meta["TT1"]
    tiles1 = meta["tiles1"]
    col1 = np.concatenate([[0], np.cumsum(tiles1)]).astype(int)
    tilesR, baseR = meta["tilesR"], meta["baseR"]
    groups, TGmax = meta["groups"], meta["TGmax"]
    colR = [np.concatenate([[0], np.cumsum(tr)]).astype(int) for tr in tilesR]
    H2 = HID // 2

    nc = bacc.Bacc("TRN2", target_bir_lowering=False, debug=False,
                   num_devices=N_CORES)
    d_edloc = nc.dram_tensor("edloc", [128, TT], F32, kind="ExternalInput")
    d_ewt = nc.dram_tensor("ewt", [128, TT], F32, kind="ExternalInput")
    d_edloc1 = nc.dram_tensor("edloc1", [128, TT1], F32, kind="ExternalInput")
    d_ewt1 = nc.dram_tensor("ewt1", [128, TT1], F32, kind="ExternalInput")
    d_xg = nc.dram_tensor("xg", [128, TT1 * IN_DIM], BF16, kind="ExternalInput")
    d_wde = nc.dram_tensor("wde", [128, TT1 * DSLOT], BF16, kind="ExternalInput")
    d_idx = nc.dram_tensor("idx16", [128, TT * 8], I16, kind="ExternalInput")
    d_xnm = nc.dram_tensor("xnm", [128, NWIN * IN_DIM], BF16, kind="ExternalInput")
    d_wdeg = nc.dram_tensor("wdeg", [128, NWIN * DSLOT], BF16, kind="ExternalInput")
    d_batch = nc.dram_tensor("batchv", [128, NWIN], F32, kind="ExternalInput")
    d_iota = nc.dram_tensor("iota", [128, 128], BF16, kind="ExternalInput")
    d_ident = nc.dram_tensor("ident", [128, 128], F32, kind="ExternalInput")
    d_identbf = nc.dram_tensor("identbf", [128, 128], BF16, kind="ExternalInput")
    d_W1 = nc.dram_tensor("W1", [IN_DIM, HID], F32, kind="ExternalInput")
    d_W2 = nc.dram_tensor("W2", [HID, HID], F32, kind="ExternalInput")
    bn_names = ["g1", "be1", "m1", "v1", "b1", "g2", "be2", "m2", "v2", "b2"]
    d_bn = {k: nc.dram_tensor(k, [1, HID], F32, kind="ExternalInput")
            for k in bn_names}
    d_lin1W = nc.dram_tensor("lin1W", [HID, H2], F32, kind="ExternalInput")
    d_lin1b = nc.dram_tensor("lin1b", [H2, 1], F32, kind="ExternalInput")
    d_lin2W = nc.dram_tensor("lin2W", [H2, 1], F32, kind="ExternalInput")
    d_lin2b = nc.dram_tensor("lin2b", [1, 1], F32, kind="ExternalInput")
    d_out = nc.dram_tensor("out", [1, 128], F32, kind="ExternalOutput")

    rg = [list(range(N_CORES))]

    with tile.TileContext(nc) as tc, ExitStack() as ctx:
        constp = ctx.enter_context(tc.tile_pool(name="const", bufs=1))
        metap = ctx.enter_context(tc.tile_pool(name="meta", bufs=1))
        wdep = ctx.enter_context(tc.tile_pool(name="wdep", bufs=3))
        msgsp = ctx.enter_context(tc.tile_pool(name="msgs", bufs=2))
        ohp = ctx.enter_context(tc.tile_pool(name="oh", bufs=12))
        ohgp = ctx.enter_context(tc.tile_pool(name="ohg", bufs=4))
        epp = ctx.enter_context(tc.tile_pool(name="ep", bufs=4))
        vecp = ctx.enter_context(tc.tile_pool(name="vec", bufs=1))
        psA = ctx.enter_context(tc.tile_pool(name="psA", bufs=2, space="PSUM"))
        ps5 = ctx.enter_context(tc.tile_pool(name="ps5", bufs=2, space="PSUM"))
        psB = ctx.enter_context(tc.tile_pool(name="psB", bufs=2, space="PSUM"))
        psPool = ctx.enter_context(tc.tile_pool(name="psP", bufs=1, space="PSUM"))
        dram = ctx.enter_context(tc.tile_pool(name="dram", bufs=1, space="DRAM"))

        nc.gpsimd.load_library(library_config.mlp)
        iota = constp.tile([128, 128], BF16)
        nc.sync.dma_start(iota[:], d_iota.ap())
        ident = constp.tile([128, 128], F32)
        nc.sync.dma_start(ident[:], d_ident.ap())
        identbf = constp.tile([128, 128], BF16)
        nc.sync.dma_start(identbf[:], d_identbf.ap())
        ones1 = constp.tile([1, 128], F32)
        nc.vector.memset(ones1[:], 1.0)

        sb_edloc = metap.tile([128, TT], F32)
        sb_ewt = metap.tile([128, TT], F32)
        sb_edloc1 = metap.tile([128, TT1], F32)
        sb_ewt1 = metap.tile([128, TT1], F32)
        nc.sync.dma_start(sb_edloc1[:], d_edloc1.ap())
        nc.sync.dma_start(sb_ewt1[:], d_ewt1.ap())
        sb_batch = metap.tile([128, NWIN], F32)
        sb_xg = metap.tile([128, TT1 * IN_DIM], BF16)
        sb_xnm = metap.tile([128, NWIN * IN_DIM], BF16)
        sb_idx = metap.tile([128, TT * 8], I16)
        nc.sync.dma_start(sb_xnm[:], d_xnm.ap())
        t2keep = metap.tile([128, NWIN * HID], BF16)
        nc.sync.dma_start(sb_edloc[:], d_edloc.ap())
        nc.sync.dma_start(sb_ewt[:], d_ewt.ap())
        nc.sync.dma_start(sb_batch[:], d_batch.ap())
        nc.sync.dma_start(sb_xg[:], d_xg.ap())
        nc.sync.dma_start(sb_idx[:], d_idx.ap())
        sb_W1 = constp.tile([IN_DIM, HID], F32)
        sb_W2 = constp.tile([HID, HID], F32)
        nc.sync.dma_start(sb_W1[:], d_W1.ap())
        nc.sync.dma_start(sb_W2[:], d_W2.ap())
        sb_bn = {}
        for k in bn_names:
            sb_bn[k] = vecp.tile([1, HID], F32, tag=k, name="sb_" + k)
            nc.sync.dma_start(sb_bn[k][:], d_bn[k].ap())
        sb_lin1W = constp.tile([HID, H2], F32)
        sb_lin1b = constp.tile([H2, 1], F32)
        sb_lin2W = constp.tile([H2, 1], F32)
        sb_lin2b = constp.tile([1, 1], F32)
        nc.sync.dma_start(sb_lin1W[:], d_lin1W.ap())
        nc.sync.dma_start(sb_lin1b[:], d_lin1b.ap())
        nc.sync.dma_start(sb_lin2W[:], d_lin2W.ap())
        nc.sync.dma_start(sb_lin2b[:], d_lin2b.ap())

        # BN folds: bns = gamma*rsqrt(var+eps); c = bns*(b - mean) + beta
        def bn_fold(g, be, m, v, b):
            bns = vecp.tile([1, HID], F32, tag="bns" + g, name="bns" + g)
            nc.vector.tensor_scalar(out=bns[:], in0=sb_bn[v][:], scalar1=EPS,
                                    scalar2=None, op0=AL.add)
            nc.scalar.activation(bns[:], bns[:], ACTF.Sqrt)
            nc.vector.reciprocal(bns[:], bns[:])
            nc.vector.tensor_tensor(out=bns[:], in0=bns[:], in1=sb_bn[g][:],
                                    op=AL.mult)
            cc = vecp.tile([1, HID], F32, tag="c" + g, name="c" + g)
            nc.vector.tensor_tensor(out=cc[:], in0=sb_bn[b][:], in1=sb_bn[m][:],
                                    op=AL.subtract)
            nc.vector.tensor_tensor(out=cc[:], in0=cc[:], in1=bns[:], op=AL.mult)
            nc.vector.tensor_tensor(out=cc[:], in0=cc[:], in1=sb_bn[be][:],
                                    op=AL.add)
            return bns, cc

        bns1, c1v = bn_fold("g1", "be1", "m1", "v1", "b1")
        bns2, c2v = bn_fold("g2", "be2", "m2", "v2", "b2")

        def bcast128(vec, tag):
            ps = psB.tile([128, HID], F32, tag="B", name="bc" + tag)
            nc.tensor.matmul(out=ps[:], lhsT=ones1[:], rhs=vec[:],
                             start=True, stop=True)
            sb = constp.tile([128, HID], F32, tag=tag, name="sb" + tag)
            nc.vector.tensor_copy(sb[:], ps[:])
            return sb

        c1_b = bcast128(c1v, "c1b")
        c2_b = bcast128(c2v, "c2b")

        def wfold(sb_W, bns, parts, tag):
            one_r = constp.tile([1, parts], F32, tag="oner" + tag,
                                name="oner" + tag)
            nc.vector.memset(one_r[:], 1.0)
            ps = psB.tile([parts, HID], F32, tag="B", name="wf" + tag)
            nc.tensor.matmul(out=ps[:], lhsT=one_r[:], rhs=bns[:],
                             start=True, stop=True)
            wp = constp.tile([parts, HID], F32, tag="wp" + tag, name="wp" + tag)
            nc.vector.tensor_tensor(out=wp[:], in0=sb_W[:], in1=ps[:], op=AL.mult)
            return wp

        W1p = wfold(sb_W1, bns1, IN_DIM, "1")
        W2p = wfold(sb_W2, bns2, HID, "2")

        t2_sh = dram.tile([SH, HID], BF16)
        t2_full = dram.tile([N + NPAR, HID], BF16)
        t2flat = t2_full[:].rearrange("n h -> (n h)")
        NROW4 = N // NPAR
        t2vr = [t2flat[r * HID: r * HID + NROW4 * NPAR * HID]
                .rearrange("(m k) -> m k", k=NPAR * HID)
                for r in range(NPAR)]
        zrow = constp.tile([NPAR, HID], BF16)
        nc.vector.memset(zrow[:], 0.0)

        dinv = constp.tile([128, NWIN], F32)
        wprime = constp.tile([128, TT1], F32)
        negwp = constp.tile([128, TT1], F32)
        negdloc1 = constp.tile([128, TT1], F32)
        negdloc = constp.tile([128, TT], F32)
        negewt = constp.tile([128, TT], F32)
        sb_wdeg = metap.tile([128, NWIN * DSLOT], BF16)

        for rep in range(reps):
            nc.sync.dma_start(t2_full[N:N + NPAR, :], zrow[:])

            # own-shard degree -> dinv_d [128, NWIN]
            nc.sync.dma_start(sb_wdeg[:], d_wdeg.ap())
            nc.vector.tensor_reduce(
                out=dinv[:].rearrange("p (j s) -> p j s", s=1),
                in_=sb_wdeg[:].rearrange("p (j s) -> p j s", s=DSLOT),
                op=AL.add, axis=mybir.AxisListType.X)
            nc.scalar.activation(dinv[:], dinv[:], ACTF.Sqrt)
            nc.vector.reciprocal(dinv[:], dinv[:])

            # per-edge w' = ewt * rsqrt(deg[src]) via chunked wde reduction
            CH = 64
            for c0 in range(0, TT1, CH):
                cw = min(CH, TT1 - c0)
                wchunk = wdep.tile([128, CH * DSLOT], BF16, tag="wde",
                                   name="wchunk")
                nc.sync.dma_start(wchunk[:, :cw * DSLOT],
                                  d_wde.ap()[:, c0 * DSLOT:(c0 + cw) * DSLOT])
                nc.vector.tensor_reduce(
                    out=wprime[:, c0:c0 + cw].rearrange("p (j s) -> p j s", s=1),
                    in_=wchunk[:, :cw * DSLOT].rearrange("p (j s) -> p j s",
                                                         s=DSLOT),
                    op=AL.add, axis=mybir.AxisListType.X)
                nc.scalar.activation(wprime[:, c0:c0 + cw], wprime[:, c0:c0 + cw],
                                     ACTF.Sqrt)
                nc.vector.reciprocal(wprime[:, c0:c0 + cw], wprime[:, c0:c0 + cw])
                nc.vector.tensor_tensor(out=wprime[:, c0:c0 + cw],
                                        in0=wprime[:, c0:c0 + cw],
                                        in1=sb_ewt1[:, c0:c0 + cw], op=AL.mult)

            nc.vector.tensor_scalar(out=negdloc[:], in0=sb_edloc[:],
                                    scalar1=-1.0, scalar2=None, op0=AL.mult)
            nc.vector.tensor_scalar(out=negdloc1[:], in0=sb_edloc1[:],
                                    scalar1=-1.0, scalar2=None, op0=AL.mult)
            nc.vector.tensor_scalar(out=negewt[:], in0=sb_ewt[:],
                                    scalar1=-1.0, scalar2=None, op0=AL.mult)
            nc.vector.tensor_scalar(out=negwp[:], in0=wprime[:],
                                    scalar1=-1.0, scalar2=None, op0=AL.mult)

            def build_diag(j):
                dg = ohp.tile([128, 128], BF16, tag="oh", name="dg")
                nc.vector.tensor_scalar(
                    out=dg[:], in0=ident[:], scalar1=dinv[:, j:j + 1],
                    scalar2=None, op0=AL.mult)
                return dg

            def oh_into(dst, col, dloctile, negdloctile, wtile, negwtile,
                        on_act):
                if on_act:
                    tt = ohp.tile([128, 128], BF16, tag="att", name="att")
                    nc.scalar.activation(tt[:], iota[:], ACTF.Abs,
                                         bias=negdloctile[:, col:col + 1])
                    nc.scalar.activation(dst, tt[:], ACTF.Relu,
                                         bias=wtile[:, col:col + 1],
                                         scale=negwtile[:, col:col + 1])
                else:
                    nc.vector.tensor_scalar(
                        out=dst, in0=iota[:],
                        scalar1=dloctile[:, col:col + 1],
                        scalar2=wtile[:, col:col + 1],
                        op0=AL.is_equal, op1=AL.mult)

            def build_onehot(col, on_act=False):
                oh = ohp.tile([128, 128], BF16, tag="oh", name="oh")
                oh_into(oh[:], col, sb_edloc1, negdloc1, wprime, negwp, on_act)
                return oh

            # L1
            for j in range(NWIN):
                wlen = min(WIN, SH - j * WIN)
                acc5 = ps5.tile([IN_DIM, 128], F32, tag="acc5", name="acc5")
                cols = [int(col1[j]) + t for t in range(tiles1[j])]
                for k, col in enumerate(cols):
                    oh = build_onehot(col, on_act=(k % 7 == 6))
                    nc.tensor.matmul(
                        out=acc5[:],
                        lhsT=sb_xg[:, col * IN_DIM:(col + 1) * IN_DIM],
                        rhs=oh[:], start=(k == 0), stop=False)
                dg1 = build_diag(j)
                nc.tensor.matmul(
                    out=acc5[:],
                    lhsT=sb_xnm[:, j * IN_DIM:(j + 1) * IN_DIM],
                    rhs=dg1[:], start=False, stop=True)
                agg5 = epp.tile([IN_DIM, 128], F32, tag="agg5", name="agg5")
                nc.vector.tensor_copy(agg5[:], acc5[:])
                ps1 = psB.tile([128, HID], F32, tag="B", name="ps1")
                nc.tensor.matmul(out=ps1[:], lhsT=agg5[:], rhs=W1p[:],
                                 start=True, stop=True)
                h1 = epp.tile([128, HID], F32, tag="h1", name="h1")
                nc.vector.scalar_tensor_tensor(
                    out=h1[:], in0=ps1[:], scalar=dinv[:, j:j + 1],
                    in1=c1_b[:], op0=AL.mult, op1=AL.add)
                nc.scalar.activation(h1[:], h1[:], ACTF.Relu)
                pT = psB.tile([HID, 128], F32, tag="B", name="pT")
                nc.tensor.transpose(out=pT[:], in_=h1[:], identity=ident[:])
                h1T = epp.tile([HID, 128], F32, tag="h1T", name="h1T")
                nc.vector.tensor_copy(h1T[:], pT[:])
                ps2 = psB.tile([128, HID], F32, tag="B", name="ps2")
                nc.tensor.matmul(out=ps2[:], lhsT=h1T[:], rhs=W2p[:],
                                 start=True, stop=True)
                # fold dinv into the table row; write bf16 into padded layout
                nc.vector.tensor_scalar(
                    out=t2keep[:, j * HID:(j + 1) * HID], in0=ps2[:],
                    scalar1=dinv[:, j:j + 1], scalar2=None, op0=AL.mult)
                nc.sync.dma_start(t2_sh[j * WIN:j * WIN + wlen, :],
                                  t2keep[:wlen, j * HID:(j + 1) * HID])



            if not no_collectives:
                nc.gpsimd.collective_compute(
                    "AllGather", AL.bypass, replica_groups=rg,
                    ins=[t2_sh[:]], outs=[t2_full[0:N, :]])

            # L2: bulk parity gathers + one-hot scatter + pooling
            pool_ps = psPool.tile([128, HID + 1], F32)
            for (g0, g1, rr) in groups:
                nR = [b - a for (a, b) in rr]
                gbase = np.concatenate([[0], np.cumsum(nR)]).astype(int)
                GT = int(gbase[-1])
                msgsG = msgsp.tile([128, sum(TGmax) * 128], BF16, tag="mG",
                                   name="msgsG")
                for r in range(NPAR):
                    a, b = rr[r]
                    if a == b:
                        continue
                    mslice = msgsG[:, int(gbase[r]) * 128:int(gbase[r + 1]) * 128]
                    if no_gather:
                        nc.vector.memset(mslice, 0.0)
                    else:
                        nc.gpsimd.dma_gather(
                            out_ap=mslice.rearrange("p (t h) -> p t h", h=128),
                            in_ap=t2vr[r][:, 0:128],
                            idxs_ap=sb_idx[:, (baseR[r] + a) * 8:
                                           (baseR[r] + b) * 8],
                            num_idxs=(b - a) * 128, num_idxs_reg=(b - a) * 128,
                            elem_size=128, elem_step=2 * 128,
                            single_packet=False)
                ohG = ohgp.tile([128, sum(TGmax) * 128], BF16,
                                tag="ohG", name="ohG")
                for j in range(g0, g1):
                    for r in range(NPAR):
                        a, _ = rr[r]
                        for t in range(tilesR[r][j]):
                            col = baseR[r] + int(colR[r][j]) + t
                            k = int(gbase[r]) + int(colR[r][j]) - a + t
                            oh_into(ohG[:, k * 128:(k + 1) * 128], col,
                                    sb_edloc, negdloc, sb_ewt, negewt,
                                    on_act=(k % 5 == 4))
                for j in range(g0, g1):
                    acc = psA.tile([128, HID], F32, tag="acc", name="acc")
                    first = True
                    for r in range(NPAR):
                        a, _ = rr[r]
                        for t in range(tilesR[r][j]):
                            k = int(gbase[r]) + int(colR[r][j]) - a + t
                            mc = k * 128
                            nc.tensor.matmul(out=acc[:],
                                             lhsT=ohG[:, mc:mc + 128],
                                             rhs=msgsG[:, mc:mc + HID],
                                             start=first, stop=False)
                            first = False
                    nc.tensor.matmul(out=acc[:], lhsT=identbf[:],
                                     rhs=t2keep[:, j * HID:(j + 1) * HID],
                                     start=False, stop=True)
                    h2e = epp.tile([128, HID + 1], BF16, tag="h2e", name="h2e")
                    nc.vector.scalar_tensor_tensor(
                        out=h2e[:, :HID], in0=acc[:], scalar=dinv[:, j:j + 1],
                        in1=c2_b[:], op0=AL.mult, op1=AL.add)
                    nc.scalar.activation(h2e[:, :HID], h2e[:, :HID], ACTF.Relu)
                    nc.vector.memset(h2e[:, HID:], 1.0)
                    ohb = ohp.tile([128, 128], BF16, tag="ohb", name="ohb")
                    nc.vector.tensor_scalar(out=ohb[:], in0=iota[:],
                                            scalar1=sb_batch[:, j:j + 1],
                                            scalar2=None, op0=AL.is_equal)
                    nc.tensor.matmul(out=pool_ps[:], lhsT=ohb[:], rhs=h2e[:],
                                     start=(j == 0), stop=(j == NWIN - 1),
                                     skip_group_check=True)

            pool_sb = epp.tile([128, HID + 1], F32, tag="poolsb", name="pool_sb")
            nc.vector.tensor_copy(pool_sb[:], pool_ps[:])
            ar_in = dram.tile([128, HID + 1], F32)
            ar_out = dram.tile([128, HID + 1], F32)
            nc.sync.dma_start(ar_in[:], pool_sb[:])
            if no_collectives:
                nc.sync.dma_start(ar_out[:], ar_in[:])
            else:
                nc.gpsimd.collective_compute(
                    "AllReduce", AL.add, replica_groups=rg,
                    ins=[ar_in.opt()], outs=[ar_out.opt()])
            sums = epp.tile([128, HID + 1], F32, tag="sums", name="sums")
            nc.sync.dma_start(sums[:], ar_out[:])

            cntc = epp.tile([128, 1], F32, tag="cnt", name="cntc")
            nc.vector.tensor_scalar(out=cntc[:], in0=sums[:, HID:HID + 1],
                                    scalar1=1.0, scalar2=None, op0=AL.max)
            rc = epp.tile([128, 1], F32, tag="rc", name="rc")
            nc.vector.reciprocal(rc[:], cntc[:])
            pooled = epp.tile([128, HID], F32, tag="pooled", name="pooled")
            nc.vector.tensor_scalar(out=pooled[:], in0=sums[:, :HID],
                                    scalar1=rc[:, :1], scalar2=None, op0=AL.mult)
            pT2 = psB.tile([HID, 128], F32, tag="B", name="pT2")
            nc.tensor.transpose(out=pT2[:], in_=pooled[:], identity=ident[:])
            pooledT = epp.tile([HID, 128], F32, tag="pooledT", name="pooledT")
            nc.vector.tensor_copy(pooledT[:], pT2[:])
            zps = psB.tile([H2, 128], F32, tag="B", name="zps")
            nc.tensor.matmul(out=zps[:], lhsT=sb_lin1W[:], rhs=pooledT[:],
                             start=True, stop=True)
            zT = epp.tile([H2, 128], F32, tag="zT", name="zT")
            nc.scalar.activation(zT[:], zps[:], ACTF.Relu, bias=sb_lin1b[:, :1])
            ops = psB.tile([1, 128], F32, tag="B", name="ops")
            nc.tensor.matmul(out=ops[:], lhsT=sb_lin2W[:], rhs=zT[:],
                             start=True, stop=True)
            outsb = epp.tile([1, 128], F32, tag="outsb", name="outsb")
            nc.vector.tensor_scalar(out=outsb[:], in0=ops[:],
                                    scalar1=sb_lin2b[:, :1], scalar2=None,
                                    op0=AL.add)
            nc.sync.dma_start(d_out.ap(), outsb[:])

    nc.compile()
    return nc


_CACHE = {}


def kernel(**inputs) -> np.ndarray:
    in_maps, meta = _prep_inputs(inputs)
    key = (meta["N"], meta["TT"], meta["TT1"], meta["DSLOT"],
           tuple(tuple(tr) for tr in meta["tilesR"]), tuple(meta["tiles1"]))
    if key not in _CACHE:
        _CACHE[key] = _build_nc(meta)
    nc = _CACHE[key]
    res = run_bass_kernel_spmd(nc, in_maps, core_ids=list(range(N_CORES)))
    out = np.asarray(res.results[0]["out"], np.float32).reshape(-1)
    return out[:meta["G"]].copy()
